# revision 9
# baseline (speedup 1.0000x reference)
"""Trainium2 Bass kernel for nn_Attn_34754875359623.

Computation (B=32, S=4096, H=256):
    scores[b,s] = u[b,s,:] @ w_eff + const(b)        (softmax is shift-invariant,
    attn        = softmax(scores, axis=s)             so const(b) is dropped)
    d[b,s]      = ||u[b,s,:] - v[b,:]||_2  = sqrt(usq - 2*u.v + vsq)
    w_d[b]      = sum_s d[b,s] * attn[b,s]
    returns (w_d, attn)

where w_eff = W_attn[:, :H].T @ v_param (host precompute, tiny).

Strategy (data-parallel over batch, 4 samples per core on 8 cores):
  - SWDGE DMA casts u fp32 -> bf16 into a DRAM scratch, chunk-major.
  - Big DMA xbar transposes (DRAM [2048,128] -> SBUF [128,2048]) put h on
    partitions. (Small SBUF-sourced transposes serialize at ~1.2us each on the
    issuing engine - measured - so the DRAM round trip is worth it.)
  - TensorE contracts over h with the tiny param matrix as the stationary
    operand and u^T streaming 512 columns per matmul: psum rows {0,1} get
    {scores, u.v}; row 32 gets usq from a ones-vector matmul against u^T**2.
  - Results ([3, S] per sample) bounce through DRAM to be re-laid-out as
    [128 partitions, ...] for the softmax/weighted-sum epilogue.
"""

import numpy as np
import ml_dtypes

B, S, H = 32, 4096, 256
NCORES = 8
BL = B // NCORES  # samples per core
T = 32            # s = j*32 + t   (j = partition, t = free) in the epilogue
HC = 2            # h chunks of 128
SH = S // 2       # transpose grain along s
QS = 1024         # psum quarter size (columns)
NBLK = QS // 512  # 512-column matmul blocks per quarter

_CACHE = {}


def _build_nc():
    from contextlib import ExitStack

    import concourse.bass as bass
    import concourse.bacc as bacc
    import concourse.tile as tile
    from concourse import mybir

    f32 = mybir.dt.float32
    bf16 = mybir.dt.bfloat16
    AF = mybir.ActivationFunctionType
    ALU = mybir.AluOpType

    nc = bacc.Bacc("TRN2", target_bir_lowering=False, debug=False)

    u_d = nc.dram_tensor("u", [BL, S, H], f32, kind="ExternalInput")
    # params[h, b, c, 0] = w_eff[c*128+h], params[h, b, c, 1] = v[b, c*128+h]
    par_d = nc.dram_tensor("par", [128, BL, HC, 2], bf16, kind="ExternalInput")
    vsq_d = nc.dram_tensor("vsq", [1, BL], f32, kind="ExternalInput")
    attn_d = nc.dram_tensor("attn", [BL, S], f32, kind="ExternalOutput")
    wd_d = nc.dram_tensor("wd", [1, BL], f32, kind="ExternalOutput")

    with tile.TileContext(nc) as tc, ExitStack() as ctx:
        singles = ctx.enter_context(tc.tile_pool(name="singles", bufs=1))
        ut_pool = ctx.enter_context(tc.tile_pool(name="ut", bufs=2))
        stage_pool = ctx.enter_context(tc.tile_pool(name="stage", bufs=2))
        small = ctx.enter_context(tc.tile_pool(name="small", bufs=2))
        psum_pool = ctx.enter_context(tc.tile_pool(name="ps", bufs=3, space="PSUM"))
        tail_psum = ctx.enter_context(tc.tile_pool(name="tps", bufs=1, space="PSUM"))
        dram_pool = ctx.enter_context(tc.tile_pool(name="dram", bufs=1, space="DRAM"))

        # --- constants / params ---
        par_sb = singles.tile([128, BL, HC, 2], bf16)
        nc.gpsimd.dma_start(out=par_sb[:], in_=par_d.ap())
        vsqb = singles.tile([128, BL], f32)
        vsq_bcast = bass.AP(
            tensor=vsq_d.ap().tensor, offset=0, ap=[[0, 128], [1, BL]]
        )
        nc.gpsimd.dma_start(out=vsqb[:], in_=vsq_bcast)
        ones_bf = singles.tile([128, 1], bf16)
        nc.vector.memset(ones_bf[:], 1.0)
        ones_col = singles.tile([128, 1], f32)
        nc.vector.memset(ones_col[:], 1.0)
        ones_row = singles.tile([1, 128], f32)
        nc.vector.memset(ones_row[:], 1.0)

        # --- DRAM scratch ---
        u16 = dram_pool.tile([BL, HC, S, 128], bf16)   # chunk-major bf16 copy of u
        scb = dram_pool.tile([BL, 3, S], f32)          # {scores, uv, usq} bounce

        relay = singles.tile([128, BL, 3, T], f32)
        e_all = singles.tile([128, BL, T], f32)
        d2 = singles.tile([128, BL, T], f32)

        for b in range(BL):
            # 1. cast u[b] fp32 -> bf16 DRAM scratch, per h-chunk (SWDGE)
            u_b = u_d.ap()[b].rearrange("s (c k) -> c s k", c=HC)
            for c in range(HC):
                nc.gpsimd.dma_start(out=u16[b, c], in_=u_b[c])

            # 2. big xbar transposes: [2048 s, 128 h] -> [128 h, 2048 s]
            ut = [[ut_pool.tile([128, SH], bf16, tag=f"ut{c}{sh}", name=f"ut{c}{sh}")
                   for sh in range(2)] for c in range(HC)]
            for sh in range(2):
                for c in range(HC):
                    nc.sync.dma_start_transpose(
                        out=ut[c][sh][:],
                        in_=u16[b, c, sh * SH:(sh + 1) * SH, :],
                    )

            # 3. squares for usq (DVE, one big op per chunk)
            u2 = [[ut_pool.tile([128, SH], bf16, tag=f"u2{c}{sh}", name=f"u2{c}{sh}")
                   for sh in range(2)] for c in range(HC)]
            for sh in range(2):
                for c in range(HC):
                    nc.vector.tensor_mul(u2[c][sh][:], ut[c][sh][:], ut[c][sh][:])

            # 4/5. matmuls (weight-batched, back-to-back) + eviction per quarter
            stage = stage_pool.tile([33, S], f32, tag="stage")
            for q in range(S // QS):
                ps = psum_pool.tile([33, QS], f32, tag="ps")
                sh, qq = divmod(q, S // QS // 2)
                for c in range(HC):
                    for blk in range(NBLK):
                        sl = slice(qq * QS + blk * 512, qq * QS + (blk + 1) * 512)
                        po = slice(blk * 512, (blk + 1) * 512)
                        nc.tensor.matmul(
                            ps[0:2, po],
                            lhsT=par_sb[:, b, c, :],
                            rhs=ut[c][sh][:, sl],
                            start=(c == 0),
                            stop=(c == HC - 1),
                        )
                for c in range(HC):
                    for blk in range(NBLK):
                        sl = slice(qq * QS + blk * 512, qq * QS + (blk + 1) * 512)
                        po = slice(blk * 512, (blk + 1) * 512)
                        nc.tensor.matmul(
                            ps[32:33, po],
                            lhsT=ones_bf[:],
                            rhs=u2[c][sh][:, sl],
                            start=(c == 0),
                            stop=(c == HC - 1),
                        )
                nc.scalar.copy(stage[:, q * QS:(q + 1) * QS], ps[:])

            # 6. bounce {scores, uv, usq} via DRAM and re-layout to [128, ...]
            nc.gpsimd.dma_start(out=scb[b, 0:2], in_=stage[0:2, :])
            nc.gpsimd.dma_start(out=scb[b, 2:3], in_=stage[32:33, :])
            relay_in = bass.AP(
                tensor=scb[:].tensor,
                offset=scb[:].offset + b * 3 * S,
                ap=[[T, 128], [S, 3], [1, T]],
            )
            nc.scalar.dma_start(out=relay[:, b], in_=relay_in)
            # d2 = usq - 2*uv (per sample, pipelined; DVE)
            nc.vector.scalar_tensor_tensor(
                out=d2[:, b], in0=relay[:, b, 1, :], scalar=-2.0,
                in1=relay[:, b, 2, :], op0=ALU.mult, op1=ALU.add,
            )

        # --- tail: softmax + weighted sum, batched over samples ---
        nc.scalar.activation(e_all[:], relay[:, :, 0, :], AF.Exp)
        vsq_b = bass.AP(
            tensor=vsqb[:].tensor, offset=vsqb[:].offset,
            ap=[[vsqb[:].ap[0][0], 128], [1, BL], [0, T]],
        )
        nc.vector.tensor_add(d2[:], d2[:], vsq_b)
        d_all = singles.tile([128, BL, T], f32)
        nc.scalar.activation(d_all[:], d2[:], AF.Sqrt)
        p_all = singles.tile([128, BL, T], f32)
        nc.vector.tensor_mul(p_all[:], d_all[:], e_all[:])
        zw = singles.tile([128, 2, BL], f32)
        nc.vector.tensor_reduce(
            zw[:, 0], e_all[:], axis=mybir.AxisListType.X, op=ALU.add
        )
        nc.vector.tensor_reduce(
            zw[:, 1], p_all[:], axis=mybir.AxisListType.X, op=ALU.add
        )
        zps = tail_psum.tile([1, 2 * BL], f32, tag="zps")
        nc.tensor.matmul(
            zps[:], lhsT=ones_col[:], rhs=zw[:].rearrange("p q b -> p (q b)")
        )
        zs = small.tile([1, 2 * BL], f32, tag="zs")
        nc.vector.tensor_copy(zs[:], zps[:])
        rz = small.tile([1, BL], f32, tag="rz")
        nc.vector.reciprocal(rz[:], zs[:, 0:BL])
        wd_sb = small.tile([1, BL], f32, tag="wd")
        nc.vector.tensor_mul(wd_sb[:], zs[:, BL:2 * BL], rz[:])
        nc.scalar.dma_start(out=wd_d.ap(), in_=wd_sb[:])

        # broadcast 1/Z to all partitions: [128, BL] = ones_row.T @ rz
        rzb_ps = tail_psum.tile([128, BL], f32, tag="rzb")
        nc.tensor.matmul(rzb_ps[:], lhsT=ones_row[:], rhs=rz[:])
        rzb = small.tile([128, BL], f32, tag="rzbs")
        nc.vector.tensor_copy(rzb[:], rzb_ps[:])
        attn_sb = singles.tile([128, BL, T], f32)
        for b in range(BL):
            nc.vector.tensor_scalar_mul(
                out=attn_sb[:, b, :], in0=e_all[:, b, :], scalar1=rzb[:, b:b + 1]
            )
        attn_out = attn_d.ap().rearrange("b (j t) -> j b t", t=T)
        nc.scalar.dma_start(out=attn_out, in_=attn_sb[:])

    nc.compile()
    return nc


def _get_nc():
    if "nc" not in _CACHE:
        _CACHE["nc"] = _build_nc()
    return _CACHE["nc"]


def _make_in_maps(u, v, W_attn, b_attn, v_param):
    bf16 = ml_dtypes.bfloat16
    u = np.ascontiguousarray(np.asarray(u, dtype=np.float32))
    v = np.asarray(v, dtype=np.float32)
    W_attn = np.asarray(W_attn, dtype=np.float32)
    v_param = np.asarray(v_param, dtype=np.float32)

    # w_eff[h] = sum_k W_attn[k, h] * v_param[k]  (the Wu = W_attn[:, :H] part)
    w_eff = (W_attn[:, :H].astype(np.float64) * v_param[:, None].astype(np.float64)).sum(axis=0)
    w_eff16 = w_eff.astype(np.float32).astype(bf16)
    v16 = v.astype(bf16)
    vsq = (v.astype(np.float64) ** 2).sum(axis=1).astype(np.float32)  # [B]

    in_maps = []
    for core in range(NCORES):
        b0 = core * BL
        par = np.empty((128, BL, HC, 2), dtype=bf16)
        for c in range(HC):
            par[:, :, c, 0] = w_eff16[c * 128:(c + 1) * 128][:, None]
            par[:, :, c, 1] = v16[b0:b0 + BL, c * 128:(c + 1) * 128].T
        in_maps.append({
            "u": u[b0:b0 + BL],
            "par": par,
            "vsq": vsq[b0:b0 + BL].reshape(1, BL),
        })
    return in_maps


def _run(in_maps, trace=False, **kwargs):
    from concourse import bass_utils
    nc = _get_nc()
    return bass_utils.run_bass_kernel_spmd(
        nc, in_maps, core_ids=list(range(NCORES)), trace=trace, **kwargs
    )


def kernel(u, v, W_attn, b_attn, v_param):
    res = _run(_make_in_maps(u, v, W_attn, b_attn, v_param))
    attn = np.concatenate([r["attn"] for r in res.results], axis=0)
    w_d = np.concatenate([r["wd"][0] for r in res.results], axis=0)
    return (w_d.astype(np.float32), attn.astype(np.float32))


# revision 10
# speedup vs baseline: 1.0152x; 1.0152x over previous
"""Trainium2 Bass kernel for nn_Attn_34754875359623.

Computation (B=32, S=4096, H=256):
    scores[b,s] = u[b,s,:] @ w_eff + const(b)        (softmax is shift-invariant,
    attn        = softmax(scores, axis=s)             so const(b) is dropped)
    d[b,s]      = ||u[b,s,:] - v[b,:]||_2  = sqrt(usq - 2*u.v + vsq)
    w_d[b]      = sum_s d[b,s] * attn[b,s]
    returns (w_d, attn)

where w_eff = W_attn[:, :H].T @ v_param (host precompute, tiny).

Strategy (data-parallel over batch, 4 samples per core on 8 cores):
  - SWDGE DMA casts u fp32 -> bf16 into a DRAM scratch, chunk-major.
  - Big DMA xbar transposes (DRAM [2048,128] -> SBUF [128,2048]) put h on
    partitions. (Small SBUF-sourced transposes serialize at ~1.2us each on the
    issuing engine - measured - so the DRAM round trip is worth it.)
  - TensorE contracts over h with the tiny param matrix as the stationary
    operand and u^T streaming 512 columns per matmul: psum rows {0,1} get
    {scores, u.v}; row 32 gets usq from a ones-vector matmul against u^T**2.
  - Results ([3, S] per sample) bounce through DRAM to be re-laid-out as
    [128 partitions, ...] for the softmax/weighted-sum epilogue.
"""

import numpy as np
import ml_dtypes

B, S, H = 32, 4096, 256
NCORES = 8
BL = B // NCORES  # samples per core
T = 32            # s = j*32 + t   (j = partition, t = free) in the epilogue
HC = 2            # h chunks of 128
SH = S // 2       # transpose grain along s
QS = 1024         # psum quarter size (columns)
NBLK = QS // 512  # 512-column matmul blocks per quarter

_CACHE = {}


def _build_nc():
    from contextlib import ExitStack

    import concourse.bass as bass
    import concourse.bacc as bacc
    import concourse.tile as tile
    from concourse import mybir

    f32 = mybir.dt.float32
    bf16 = mybir.dt.bfloat16
    AF = mybir.ActivationFunctionType
    ALU = mybir.AluOpType

    nc = bacc.Bacc("TRN2", target_bir_lowering=False, debug=False)

    u_d = nc.dram_tensor("u", [BL, S, H], f32, kind="ExternalInput")
    # params[h, b, c, 0] = w_eff[c*128+h], params[h, b, c, 1] = v[b, c*128+h]
    par_d = nc.dram_tensor("par", [128, BL, HC, 2], bf16, kind="ExternalInput")
    vsq_d = nc.dram_tensor("vsq", [1, BL], f32, kind="ExternalInput")
    attn_d = nc.dram_tensor("attn", [BL, S], f32, kind="ExternalOutput")
    wd_d = nc.dram_tensor("wd", [1, BL], f32, kind="ExternalOutput")

    with tile.TileContext(nc) as tc, ExitStack() as ctx:
        singles = ctx.enter_context(tc.tile_pool(name="singles", bufs=1))
        ut_pool = ctx.enter_context(tc.tile_pool(name="ut", bufs=2))
        stage_pool = ctx.enter_context(tc.tile_pool(name="stage", bufs=2))
        small = ctx.enter_context(tc.tile_pool(name="small", bufs=2))
        psum_pool = ctx.enter_context(tc.tile_pool(name="ps", bufs=3, space="PSUM"))
        tail_psum = ctx.enter_context(tc.tile_pool(name="tps", bufs=1, space="PSUM"))
        dram_pool = ctx.enter_context(tc.tile_pool(name="dram", bufs=1, space="DRAM"))

        # --- constants / params ---
        par_sb = singles.tile([128, BL, HC, 2], bf16)
        nc.gpsimd.dma_start(out=par_sb[:], in_=par_d.ap())
        vsqb = singles.tile([128, BL], f32)
        vsq_bcast = bass.AP(
            tensor=vsq_d.ap().tensor, offset=0, ap=[[0, 128], [1, BL]]
        )
        nc.gpsimd.dma_start(out=vsqb[:], in_=vsq_bcast)
        ones_bf = singles.tile([128, 1], bf16)
        nc.vector.memset(ones_bf[:], 1.0)
        ones_col = singles.tile([128, 1], f32)
        nc.vector.memset(ones_col[:], 1.0)
        ones_row = singles.tile([1, 128], f32)
        nc.vector.memset(ones_row[:], 1.0)

        # --- DRAM scratch (separate tiles per (b, c) so dependency tracking
        # stays precise; a single tile makes every reader wait on the last
        # writer, serializing the whole pipeline) ---
        u16 = [[dram_pool.tile([S, 128], bf16, tag=f"u16_{b}_{c}", name=f"u16_{b}_{c}")
                for c in range(HC)] for b in range(BL)]
        scb = [dram_pool.tile([3, S], f32, tag=f"scb_{b}", name=f"scb_{b}")
               for b in range(BL)]

        relay = singles.tile([128, BL, 3, T], f32)
        e_all = singles.tile([128, BL, T], f32)
        d2 = singles.tile([128, BL, T], f32)

        for b in range(BL):
            # 1. cast u[b] fp32 -> bf16 DRAM scratch, per h-chunk (SWDGE)
            u_b = u_d.ap()[b].rearrange("s (c k) -> c s k", c=HC)
            for c in range(HC):
                nc.gpsimd.dma_start(out=u16[b][c][:], in_=u_b[c])

            # 2. big xbar transposes: [2048 s, 128 h] -> [128 h, 2048 s]
            ut = [[ut_pool.tile([128, SH], bf16, tag=f"ut{c}{sh}", name=f"ut{c}{sh}")
                   for sh in range(2)] for c in range(HC)]
            for sh in range(2):
                for c in range(HC):
                    nc.sync.dma_start_transpose(
                        out=ut[c][sh][:],
                        in_=u16[b][c][sh * SH:(sh + 1) * SH, :],
                    )

            # 3. squares for usq (DVE, one big op per chunk)
            u2 = [[ut_pool.tile([128, SH], bf16, tag=f"u2{c}{sh}", name=f"u2{c}{sh}")
                   for sh in range(2)] for c in range(HC)]
            for sh in range(2):
                for c in range(HC):
                    nc.vector.tensor_mul(u2[c][sh][:], ut[c][sh][:], ut[c][sh][:])

            # 4/5. matmuls (weight-batched, back-to-back) + eviction per quarter
            stage = stage_pool.tile([33, S], f32, tag="stage")
            for q in range(S // QS):
                ps = psum_pool.tile([33, QS], f32, tag="ps")
                sh, qq = divmod(q, S // QS // 2)
                for c in range(HC):
                    for blk in range(NBLK):
                        sl = slice(qq * QS + blk * 512, qq * QS + (blk + 1) * 512)
                        po = slice(blk * 512, (blk + 1) * 512)
                        nc.tensor.matmul(
                            ps[0:2, po],
                            lhsT=par_sb[:, b, c, :],
                            rhs=ut[c][sh][:, sl],
                            start=(c == 0),
                            stop=(c == HC - 1),
                        )
                for c in range(HC):
                    for blk in range(NBLK):
                        sl = slice(qq * QS + blk * 512, qq * QS + (blk + 1) * 512)
                        po = slice(blk * 512, (blk + 1) * 512)
                        nc.tensor.matmul(
                            ps[32:33, po],
                            lhsT=ones_bf[:],
                            rhs=u2[c][sh][:, sl],
                            start=(c == 0),
                            stop=(c == HC - 1),
                        )
                nc.scalar.copy(stage[:, q * QS:(q + 1) * QS], ps[:])

            # 6. bounce {scores, uv, usq} via DRAM and re-layout to [128, ...]
            nc.gpsimd.dma_start(out=scb[b][0:2], in_=stage[0:2, :])
            nc.gpsimd.dma_start(out=scb[b][2:3], in_=stage[32:33, :])
            relay_in = bass.AP(
                tensor=scb[b][:].tensor,
                offset=scb[b][:].offset,
                ap=[[T, 128], [S, 3], [1, T]],
            )
            nc.scalar.dma_start(out=relay[:, b], in_=relay_in)
            # d2 = usq - 2*uv (per sample, pipelined; DVE)
            nc.vector.scalar_tensor_tensor(
                out=d2[:, b], in0=relay[:, b, 1, :], scalar=-2.0,
                in1=relay[:, b, 2, :], op0=ALU.mult, op1=ALU.add,
            )

        # --- tail: softmax + weighted sum, batched over samples ---
        nc.scalar.activation(e_all[:], relay[:, :, 0, :], AF.Exp)
        vsq_b = bass.AP(
            tensor=vsqb[:].tensor, offset=vsqb[:].offset,
            ap=[[vsqb[:].ap[0][0], 128], [1, BL], [0, T]],
        )
        nc.vector.tensor_add(d2[:], d2[:], vsq_b)
        d_all = singles.tile([128, BL, T], f32)
        nc.scalar.activation(d_all[:], d2[:], AF.Sqrt)
        p_all = singles.tile([128, BL, T], f32)
        nc.vector.tensor_mul(p_all[:], d_all[:], e_all[:])
        zw = singles.tile([128, 2, BL], f32)
        nc.vector.tensor_reduce(
            zw[:, 0], e_all[:], axis=mybir.AxisListType.X, op=ALU.add
        )
        nc.vector.tensor_reduce(
            zw[:, 1], p_all[:], axis=mybir.AxisListType.X, op=ALU.add
        )
        zps = tail_psum.tile([1, 2 * BL], f32, tag="zps")
        nc.tensor.matmul(
            zps[:], lhsT=ones_col[:], rhs=zw[:].rearrange("p q b -> p (q b)")
        )
        zs = small.tile([1, 2 * BL], f32, tag="zs")
        nc.vector.tensor_copy(zs[:], zps[:])
        rz = small.tile([1, BL], f32, tag="rz")
        nc.vector.reciprocal(rz[:], zs[:, 0:BL])
        wd_sb = small.tile([1, BL], f32, tag="wd")
        nc.vector.tensor_mul(wd_sb[:], zs[:, BL:2 * BL], rz[:])
        nc.scalar.dma_start(out=wd_d.ap(), in_=wd_sb[:])

        # broadcast 1/Z to all partitions: [128, BL] = ones_row.T @ rz
        rzb_ps = tail_psum.tile([128, BL], f32, tag="rzb")
        nc.tensor.matmul(rzb_ps[:], lhsT=ones_row[:], rhs=rz[:])
        rzb = small.tile([128, BL], f32, tag="rzbs")
        nc.vector.tensor_copy(rzb[:], rzb_ps[:])
        attn_sb = singles.tile([128, BL, T], f32)
        for b in range(BL):
            nc.vector.tensor_scalar_mul(
                out=attn_sb[:, b, :], in0=e_all[:, b, :], scalar1=rzb[:, b:b + 1]
            )
        attn_out = attn_d.ap().rearrange("b (j t) -> j b t", t=T)
        nc.scalar.dma_start(out=attn_out, in_=attn_sb[:])

    nc.compile()
    return nc


def _get_nc():
    if "nc" not in _CACHE:
        _CACHE["nc"] = _build_nc()
    return _CACHE["nc"]


def _make_in_maps(u, v, W_attn, b_attn, v_param):
    bf16 = ml_dtypes.bfloat16
    u = np.ascontiguousarray(np.asarray(u, dtype=np.float32))
    v = np.asarray(v, dtype=np.float32)
    W_attn = np.asarray(W_attn, dtype=np.float32)
    v_param = np.asarray(v_param, dtype=np.float32)

    # w_eff[h] = sum_k W_attn[k, h] * v_param[k]  (the Wu = W_attn[:, :H] part)
    w_eff = (W_attn[:, :H].astype(np.float64) * v_param[:, None].astype(np.float64)).sum(axis=0)
    w_eff16 = w_eff.astype(np.float32).astype(bf16)
    v16 = v.astype(bf16)
    vsq = (v.astype(np.float64) ** 2).sum(axis=1).astype(np.float32)  # [B]

    in_maps = []
    for core in range(NCORES):
        b0 = core * BL
        par = np.empty((128, BL, HC, 2), dtype=bf16)
        for c in range(HC):
            par[:, :, c, 0] = w_eff16[c * 128:(c + 1) * 128][:, None]
            par[:, :, c, 1] = v16[b0:b0 + BL, c * 128:(c + 1) * 128].T
        in_maps.append({
            "u": u[b0:b0 + BL],
            "par": par,
            "vsq": vsq[b0:b0 + BL].reshape(1, BL),
        })
    return in_maps


def _run(in_maps, trace=False, **kwargs):
    from concourse import bass_utils
    nc = _get_nc()
    return bass_utils.run_bass_kernel_spmd(
        nc, in_maps, core_ids=list(range(NCORES)), trace=trace, **kwargs
    )


def kernel(u, v, W_attn, b_attn, v_param):
    res = _run(_make_in_maps(u, v, W_attn, b_attn, v_param))
    attn = np.concatenate([r["attn"] for r in res.results], axis=0)
    w_d = np.concatenate([r["wd"][0] for r in res.results], axis=0)
    return (w_d.astype(np.float32), attn.astype(np.float32))
